# revision 47
# baseline (speedup 1.0000x reference)
"""GAT layer (B=4, N=2048, D=256, H=4) on 8 trn2 NeuronCores.

Sharding: core c -> (b = c//2, i-half = c%2).  Each core computes
out[b, ihalf*1024:(ihalf+1)*1024, :]; h is computed on-device from the full
x[b] (passed pre-transposed as x[b].T, bf16).

Math: with z = s_src[i] + s_dst[j], the reference computes
    alpha = softmax_j(mask(leaky_relu(z)));  out = alpha @ h_head.
Softmax is shift-invariant per destination row i, so we use shifted
unnormalized weights (same alpha):
    P[i,j] = adj[i,j] * exp(leaky_relu(z) - s_src[i])
           = adj[i,j] * max(F1[j], E2[i] * J[j])
with F1 = exp(s_dst), J = exp(0.2*s_dst), E2 = exp(-0.8*s_src)
(z>=0 branch gives exp(s_dst)=F1; z<0 branch gives exp(0.2z-s_src)=E2*J).
Row sums come from an appended ones-column in the aggregation matmul
stationary:  psoT = [h_head | 1].T @ P^T  ->  out = psoT[:64] / psoT[64].

v3 schedule (HW-measured, 105.9us -> ~92.7us): the DVE elementwise stream
(per jt-pair-iter: 2x tensor_scalar @535 + 1x 2-head tensor_tensor @1233,
~2.1us x 32) is the pace-setter; everything else must hide under it.
  - startup (first TS 22us -> ~16us): merged single-issue DMAs (each
    dma_start costs ~650ns of queue-engine time), the three critical loads
    (xit kt0 / xit kt1 / wta) on three different queues (sync/gpsimd/
    scalar), pss->er4->e2rep emitted before h_aug, jf0/jf1 via tiny 8-col
    matmuls instead of the full h_aug, and 9x 512-col dummy matmuls to
    flip the HAM clock gate (1.2->2.4GHz) while the DMAs land.
  - tail (14us -> ~6us): pair-1 epilogues batch soT copies (ACT+DVE in
    parallel), stream the 16 fp32 transposes back-to-back on PE, normalize
    on DVE (idle in the tail), one merged output DMA per i-half.
  - hplus ones-columns via a tiny strided DVE memset (the 3.6us full-tile
    gpsimd memset blocked the gpsimd DMA queue).

MEASUREMENT TRAP: the chip has a bistable P0 power state that downclocks
ALL engines x5/6 for minutes under sustained load (exec ~93us <-> ~112us
for the SAME binary; TS 535<->665, TT 1233<->1476, MM 379<->454).  Check
per-op durations for a uniform x1.2 scale before attributing a regression
to a code change; ~4min idle restores fast mode.

Measured dead ends (some possibly confounded by P0 -- re-test carefully):
  - changing the adjt pool to 8x[128,2048] paired tiles coincided with an
    ~18% DVE slowdown (kept original pool shapes to be safe).
  - merged 4D tensor_tensor (2jt x 2head): 2751ns vs 2x1233 split (5D AP
    sub-dim overhead) AND triggers the layout slowdown above.
  - J-fold into hplus (single-op max TS, 479 vs 535ns): the per-head ACT
    scale-copies push ACT to ~55us busy and the DVE stalls ~10us on jf.
  - gpsimd tensor ops ~15us per [128,1024] + starve DVE read ports;
    scalar_tensor_tensor runs at 1x (1276ns) so fusing TS+TT loses.
  - fp8 aggregation fails accuracy (~2x the 2e-2 budget); 1024-col
    matmuls rejected by walrus (psum bank = 512 fp32); AF.Reciprocal
    blocked by bass (accuracy); 4-way split output DMA slightly worse.
"""

import sys

for _p in ("/opt/trn_rl_repo", "/root/.axon_site/_ro/trn_rl_repo"):
    if _p not in sys.path:
        sys.path.insert(0, _p)

import ml_dtypes
import numpy as np

import concourse.bass as bass
import concourse.mybir as mybir
from concourse import masks, tile
from concourse.bass_utils import run_bass_kernel_spmd
from concourse.vector_clock import ScopedClock

F32 = mybir.dt.float32
BF16 = mybir.dt.bfloat16
AF = mybir.ActivationFunctionType
ALU = mybir.AluOpType

B, N, D, H, HD = 4, 2048, 256, 4, 64
NEG_SLOPE = 0.2
P = 128
NI = N // 2          # i-rows per core (1024)
NT = N // P          # 16 j/n tiles
KT = D // P          # 2 k tiles
JT = NT
JQ = JT // 2         # 8 jt-pairs
ISUB = NI // P       # 8
NCORES = 8
WC = D + 3 * H       # 268 aug cols: [W.T | WtAdst | 0.2*WtAdst | WtAsrc]
HP1 = HD + 1         # 65
HPW = H * HP1        # 260 hplus cols per j-tile


def _patch_tile_drain():
    """walrus rejects >1 sem wait on one instruction in this toolchain; split
    the TileContext tail drain's waits across consecutive SP drains."""
    if getattr(tile.TileContext, "_drain_patched", False):
        return

    def _drain_and_barrier(self, tick_clock, wait_clock):
        nc = self.nc
        drain_inst = nc.sync.drain()
        wait_clock.add_sem_waits(
            drain_inst.ins, ScopedClock({None: tick_clock.global_clock})
        )
        si = drain_inst.ins.sync_info
        waits = list(si.on_wait) if (si and si.on_wait) else []
        if len(waits) > 1:
            ups = list(si.on_update) if (si and si.on_update) else []
            drain_inst.ins.sync_info = mybir.SyncInfo(on_wait=waits[:1], on_update=ups)
            for i in range(1, len(waits)):
                extra = nc.sync.drain()
                extra.ins.sync_info = mybir.SyncInfo(
                    on_wait=waits[i : i + 1], on_update=[]
                )
        nc.all_engine_barrier()
        assert self.sems is not None
        popped = nc._tile_sem_poison_stack.pop()
        assert popped is self._sem_poison
        nc.clear_and_free_semaphores(list(self.sems.allocated().values()))
        nc.all_engine_barrier()

    tile.TileContext._drain_and_barrier = _drain_and_barrier
    tile.TileContext._drain_patched = True


def _split_waits(nc, maxw=1):
    """Hoist excess sem waits onto same-engine EventSemaphore carriers placed
    just before the instruction (same engine + program order => equivalent)."""
    n_split = 0
    for f in nc.m.functions:
        for bb in f.blocks:
            insts = list(bb.instructions)
            out = []
            changed = False
            for inst in insts:
                si = inst.sync_info
                waits = list(si.on_wait) if (si and si.on_wait) else []
                if len(waits) > maxw and inst.engine is not None:
                    changed = True
                    extra, keep = waits[:-maxw], waits[-maxw:]
                    for k in range(0, len(extra), maxw):
                        d = mybir.InstEventSemaphore(
                            name=f"{inst.name}-wsplit{k}", ins=[], outs=[]
                        )
                        d.engine = inst.engine
                        d.sync_info = mybir.SyncInfo(
                            on_wait=extra[k : k + maxw], on_update=[]
                        )
                        out.append(d)
                        n_split += 1
                    ups = list(si.on_update) if (si and si.on_update) else []
                    inst.sync_info = mybir.SyncInfo(on_wait=keep, on_update=ups)
                out.append(inst)
            if changed:
                bb.instructions = out
    return n_split


def build_nc():
    _patch_tile_drain()
    nc = bass.Bass("TRN2", target_bir_lowering=False, debug=False)

    xbt = nc.dram_tensor("xbt", [D, N], BF16, kind="ExternalInput")    # x[b].T
    xit = nc.dram_tensor("xit", [D, NI], BF16, kind="ExternalInput")   # xi.T
    wta = nc.dram_tensor("wta", [D, WC], BF16, kind="ExternalInput")
    adjtb = nc.dram_tensor("adjtb", [N, NI], BF16, kind="ExternalInput")
    selm = nc.dram_tensor("selm", [H, H * P], BF16, kind="ExternalInput")
    outs = nc.dram_tensor("outs", [NI, D], F32, kind="ExternalOutput")

    with tile.TileContext(nc) as tc:
        with (
            tc.tile_pool(name="const", bufs=1) as constp,
            tc.tile_pool(name="big", bufs=1) as bigp,
            tc.tile_pool(name="rows", bufs=1) as rowsp,
            tc.tile_pool(name="jf", bufs=16) as jfp,
            tc.tile_pool(name="adjt", bufs=17) as adjtp,
            tc.tile_pool(name="vwork", bufs=3) as vp,
            tc.tile_pool(name="ptwork", bufs=5) as ptp,
            tc.tile_pool(name="sot", bufs=5) as sotp,
            tc.tile_pool(name="small", bufs=6) as smallp,
            tc.tile_pool(name="psall", bufs=1, space="PSUM") as psall,
        ):
            # transient psum tiles round-robin banks 4-7; psoT/ps2 use banks 0-3
            ps_ctr = [0]

            def ps_tile(shape, name, tag=None):
                if tag is None:
                    tag = f"bank{4 + ps_ctr[0] % 4}"
                    ps_ctr[0] += 1
                return psall.tile(shape, F32, tag=tag, name=name)

            pe_prev = [None]

            def pe(bi):
                # pin PE stream order: PSUM accumulation groups must stay
                # contiguous on PE (interleaving corrupts accumulation on HW)
                if pe_prev[0] is not None:
                    tile.add_dep_helper(bi.ins, pe_prev[0], reason="pe-order")
                pe_prev[0] = bi.ins
                return bi

            ident = constp.tile([P, P], F32, tag="ident")
            masks.make_identity(nc, ident[:])

            wta_all = constp.tile([P, KT * WC], BF16, tag="wta01", name="wta_all")

            def wta_mov(kt):
                return wta_all[:, kt * WC : (kt + 1) * WC]

            def wta_srcc(kt):
                return wta_all[:, kt * WC + D + 2 * H : kt * WC + D + 3 * H]
            sel_sb = constp.tile([H, H * P], BF16, tag="selm")
            sels = [sel_sb[:, h * P : (h + 1) * P] for h in range(H)]

            xit_sb = bigp.tile([P, KT * NI], BF16, tag="xit")
            xt_sb = bigp.tile([P, KT * N], BF16, tag="xt")
            # only the ones-columns of hplus need init (the h_head parts are
            # fully written by the ACT copies); a tiny strided DVE memset
            # keeps the 3.6us full-tile gpsimd memset off the DMA queue.
            # NOTE: folding J into hplus (single-op max TS) was tried and
            # REGRESSED +9us: the per-head ACT scale-copies push ACT to
            # ~55us busy and the DVE stalls waiting on jf tiles.
            hplus = bigp.tile([P, NT * HPW], BF16, tag="hplus")
            hp4i = hplus[:].rearrange("p (t h c) -> p t h c", t=NT, h=H)
            nc.vector.memset(hp4i[:, :, :, HD : HD + 1], 1.0)
            adjts = [
                adjtp.tile([P, NI], BF16, tag="adjt", name=f"adjt_{jt}")
                for jt in range(JT)
            ]

            # ---- DMA schedule: transfers on one queue serialize, so the two
            # queues each carry one kt-half of the critical tensors.  Order:
            # wta (small, needed by every matmul) then xit (pss chain) then
            # the first 256 xt cols (jf0/jf1 + h_aug nt0/nt1) then adj jt0/1
            # (first TT), then the bulk.
            # critical transfers on three separate queues so none serializes
            # behind another: xit kt0 on sync, xit kt1 on gpsimd, wta on
            # scalar (ACT is idle until the er4 exps need it)
            nc.sync.dma_start(xit_sb[:, 0:NI], xit[0:P, :])
            nc.gpsimd.dma_start(xit_sb[:, NI : 2 * NI], xit[P : 2 * P, :])
            nc.scalar.dma_start(
                wta_all[:].rearrange("p (k c) -> p k c", k=KT),
                wta[:, :].rearrange("(k p) c -> p k c", k=KT),
            )
            nc.gpsimd.dma_start(
                xt_sb[:, N : N + 256], xbt[P : 2 * P, 0:256])
            nc.sync.dma_start(
                xt_sb[:, 0:256], xbt[0:P, 0:256])
            nc.sync.dma_start(sel_sb[:], selm[:])
            for jt in range(2):
                nc.gpsimd.dma_start(adjts[jt][:], adjtb[jt * P : (jt + 1) * P, :])
            # NOTE: routing adj tiles through the scalar/ACT queue measured a
            # +19us regression (ACT-queue DMA transfers starve ACT work) --
            # keep adj on sync/gpsimd only.
            # bulk xt: rest of each kt-half on its own queue
            nc.sync.dma_start(
                xt_sb[:, 256:N], xbt[0:P, 256:N])
            nc.gpsimd.dma_start(
                xt_sb[:, N + 256 : 2 * N], xbt[P : 2 * P, 256:N])
            # remaining adj tiles interleave across both queues
            for jt in range(2, JT):
                eng = nc.gpsimd if jt % 2 == 0 else nc.sync
                eng.dma_start(adjts[jt][:], adjtb[jt * P : (jt + 1) * P, :])

            # hplus memset gates the per-nt strided copies; emitted before any
            # gpsimd DMA issues so it lands at t~0, not behind the DMA queue
            hp4 = hplus[:].rearrange("p (t h c) -> p t h c", t=NT, h=H)

            # HAM warmup: PE is otherwise idle until the xit DMA lands, so
            # the startup matmul chain runs at the cold 1.2GHz clock.  ~3.5us
            # of dummy matmul activity flips the HAM gate to 2.4GHz first.
            # hplus is garbage here; pss start=True clears the psum after.
            # 512-col moving so each dummy occupies the array ~427ns; 9 of
            # them span ~3.8us = a full HAM window, ending as the xit DMA
            # lands (~11.4us) so the real chain runs at 2.4GHz
            psd = ps_tile([8, 512], "psd", tag="bank4")
            for _ in range(9):
                pe(nc.tensor.matmul(
                    psd[:], hplus[0:P, 0:8], hplus[:, 0:512],
                    start=True, stop=True,
                ))

            # ---- s_srcT (all heads) -> E2 rows [4, NI] (bf16) ----
            er4 = rowsp.tile([H, NI], BF16, tag="er4")
            for c in range(NI // 512):
                pss = ps_tile([H, 512], f"pss_{c}")
                for kt in range(KT):
                    pe(nc.tensor.matmul(
                        pss[:],
                        wta_srcc(kt),
                        xit_sb[:, kt * NI + c * 512 : kt * NI + (c + 1) * 512],
                        start=(kt == 0),
                        stop=(kt == KT - 1),
                    ))
                nc.scalar.activation(
                    er4[:, c * 512 : (c + 1) * 512],
                    pss[:],
                    AF.Exp,
                    scale=-(1.0 - NEG_SLOPE),
                )
            e2rep = bigp.tile([P, H * NI], BF16, tag="e2rep")

            def emit_e2rep(h):
                for c in range(NI // 512):
                    psb = ps_tile([P, 512], f"psb_{h}_{c}")
                    pe(nc.tensor.matmul(
                        psb[:], sels[h], er4[0:H, c * 512 : (c + 1) * 512]
                    ))
                    nc.scalar.activation(
                        e2rep[:, h * NI + c * 512 : h * NI + (c + 1) * 512],
                        psb[:],
                        AF.Copy,
                    )

            # ---- h_aug = x @ wta (bf16); JF = [F1|J]; hplus strided copy ----
            jf_tiles = {}

            def emit_haug(nt):
                psh = ps_tile([P, WC], f"psh_{nt}")
                for kt in range(KT):
                    pe(nc.tensor.matmul(
                        psh[:],
                        xt_sb[:, kt * N + nt * P : kt * N + (nt + 1) * P],
                        wta_mov(kt),
                        start=(kt == 0),
                        stop=(kt == KT - 1),
                    ))
                if nt not in jf_tiles:
                    jf = jfp.tile([P, 2 * H], F32, tag="jf", name=f"jf_{nt}")
                    nc.scalar.activation(jf[:], psh[:, D : D + 2 * H], AF.Exp)
                    jf_tiles[nt] = jf
                nc.scalar.activation(
                    hp4[:, nt, :, 0:HD],
                    psh[:, 0:D].rearrange("p (h c) -> p h c", h=H),
                    AF.Copy,
                )

            def emit_jf_early(nt):
                # jf only needs the 2H score columns of psh -- a ~190ns tiny
                # matmul per kt instead of waiting on the full h_aug
                psj = ps_tile([P, 2 * H], f"psj_{nt}")
                for kt in range(KT):
                    pe(nc.tensor.matmul(
                        psj[:],
                        xt_sb[:, kt * N + nt * P : kt * N + (nt + 1) * P],
                        wta_all[:, kt * WC + D : kt * WC + D + 2 * H],
                        start=(kt == 0),
                        stop=(kt == KT - 1),
                    ))
                jf = jfp.tile([P, 2 * H], F32, tag="jf", name=f"jf_{nt}")
                nc.scalar.activation(jf[:], psj[:], AF.Exp)
                jf_tiles[nt] = jf

            # critical order for the first TS/TT of pair 0 (heads 0/1, jt 0/1):
            # pss -> e2rep h0/h1 (copies on ACT, keeping the DVE queue clear
            # for the first TS); jf0/jf1 via tiny early matmuls; the full
            # h_aug (for hplus) follows
            emit_e2rep(0)
            emit_jf_early(0)
            emit_jf_early(1)
            emit_e2rep(1)
            # pre-produce jf for nt 2..7 too: the tiny matmuls are ~2x100ns
            # on a warm PE, and pulling the exps ahead of the bulky hp/e2rep
            # ACT ops keeps the DVE from stalling on jf supply early in the
            # main loop (measured ~4us of stretched iterations there)
            for nt in range(2, 8):
                emit_jf_early(nt)
            emit_haug(0)
            emit_haug(1)
            emit_haug(2)
            emit_haug(3)
            emit_e2rep(2)
            emit_e2rep(3)
            for nt in range(4, NT):
                emit_haug(nt)

            # ---- main: P^T construction (DVE) + aggregation + epilogue ----
            ost = bigp.tile([P, ISUB * D], F32, tag="ost")
            ost8 = ost[:].rearrange("p (s c) -> p s c", s=ISUB)

            # epilogue for one (h, half) combo (pair-0 / mid-kernel style:
            # ACT-heavy, stays off the DVE critical path)
            def emit_epilogue(pair, h01, half, psoT):
                h = 2 * pair + h01
                soT = sotp.tile([HP1, 512], F32, tag="soT", name=f"soT_{h}_{half}")
                nc.scalar.activation(soT[:], psoT[:], AF.Copy)
                ps2 = psall.tile(
                    [P, H * HP1], F32, tag=f"bank{h01 * 2 + half}",
                    name=f"ps2_{h}_{half}",
                )
                for q in range(4):
                    pe(nc.tensor.transpose(
                        ps2[:, q * HP1 : (q + 1) * HP1],
                        soT[:, q * P : (q + 1) * P],
                        ident[0:HP1, 0:HP1],
                    ))
                ps2q = ps2[:].rearrange("p (q c) -> p q c", q=4)
                rec4 = smallp.tile([P, 4], F32, tag="rec", name=f"rec_{h}_{half}")
                nc.vector.reciprocal(rec4[:], ps2q[:, :, HD])
                for q in range(4):
                    nc.scalar.activation(
                        ost8[:, half * 4 + q, h * HD : (h + 1) * HD],
                        ps2q[:, q, 0:HD],
                        AF.Copy,
                        scale=rec4[:, q : q + 1],
                    )

            pending_epi = []
            for pair in range(2):
                # pair0 accumulates on banks 0-3, pair1 on banks 4-7 (the
                # h_aug/pss transients are done by then): decoupling the
                # banks lets pair0's epilogues (ps2 on banks 0-3) spread
                # across pair1's first iterations instead of wedging between
                # the pairs, where the DVE reciprocals stalled the stream.
                while pending_epi and pair == 0:
                    emit_epilogue(*pending_epi.pop(0))
                psoTs = {}
                for h01 in range(2):
                    for half in range(2):
                        psoTs[(h01, half)] = psall.tile(
                            [HP1, 512], F32,
                            tag=f"bank{4 * pair + h01 * 2 + half}",
                            name=f"psoT_{2 * pair + h01}_{half}",
                        )
                for jt in range(JT):
                    if pending_epi and jt in (2, 4, 6, 8):
                        emit_epilogue(*pending_epi.pop(0))
                    adjv = adjts[jt][:]
                    # v = max(e2rep*J, F1) per head; pt = v*adj (merged 2-head)
                    v2 = vp.tile([P, 2 * NI], BF16, tag="v", name=f"v_{pair}_{jt}")
                    for h01 in range(2):
                        h = 2 * pair + h01
                        nc.vector.tensor_scalar(
                            v2[:, h01 * NI : (h01 + 1) * NI],
                            e2rep[:, h * NI : (h + 1) * NI],
                            jf_tiles[jt][:, H + h : H + h + 1],
                            jf_tiles[jt][:, h : h + 1],
                            ALU.mult,
                            ALU.max,
                        )
                    pt2 = ptp.tile([P, 2 * NI], BF16, tag="pt", name=f"pt_{pair}_{jt}")
                    nc.vector.tensor_tensor(
                        pt2[:].rearrange("p (g c) -> p g c", g=2),
                        v2[:].rearrange("p (g c) -> p g c", g=2),
                        adjv.unsqueeze(1).broadcast_to([P, 2, NI]),
                        ALU.mult,
                    )
                    # jt-major aggregation: 4 interleaved accumulation groups
                    # (verified on HW: per-cell has_written bits make
                    # interleaved groups on different banks safe)
                    for h01 in range(2):
                        h = 2 * pair + h01
                        for half in range(2):
                            pe(nc.tensor.matmul(
                                psoTs[(h01, half)][:],
                                hplus[:, jt * HPW + h * HP1 : jt * HPW + (h + 1) * HP1],
                                pt2[:, h01 * NI + half * 512 : h01 * NI + (half + 1) * 512],
                                start=(jt == 0),
                                stop=(jt == JT - 1),
                                skip_group_check=True,
                            ))
                for half in range(2):
                    for h01 in range(2):
                        pending_epi.append((pair, h01, half, psoTs[(h01, half)]))

            # ---- tail: pair-1 epilogues, restructured for minimum latency:
            # all soT copies first (split ACT/DVE so they run in parallel),
            # then the 16 transposes stream back-to-back on PE, then DVE
            # reciprocal + DVE normalize (DVE is idle in the tail), with the
            # output DMAs interleaved per half.
            tail = [pending_epi.pop(0) for _ in range(4)]
            soTs = {}
            for k, (pair, h01, half, psoT) in enumerate(tail):
                h = 2 * pair + h01
                soT = sotp.tile(
                    [HP1, 512], F32, tag="soT", name=f"soTt_{h}_{half}"
                )
                if k % 2 == 0:
                    nc.scalar.activation(soT[:], psoT[:], AF.Copy)
                else:
                    nc.vector.tensor_copy(soT[:], psoT[:])
                soTs[(h01, half)] = soT
            ps2s = {}
            for pair, h01, half, psoT in tail:
                h = 2 * pair + h01
                ps2 = psall.tile(
                    [P, H * HP1], F32, tag=f"bank{h01 * 2 + half}",
                    name=f"ps2t_{h}_{half}",
                )
                for q in range(4):
                    pe(nc.tensor.transpose(
                        ps2[:, q * HP1 : (q + 1) * HP1],
                        soTs[(h01, half)][:, q * P : (q + 1) * P],
                        ident[0:HP1, 0:HP1],
                    ))
                ps2s[(h01, half)] = ps2[:].rearrange("p (q c) -> p q c", q=4)
            done_half = set()
            for pair, h01, half, psoT in tail:
                h = 2 * pair + h01
                ps2q = ps2s[(h01, half)]
                rec4 = smallp.tile([P, 4], F32, tag="rec", name=f"rect_{h}_{half}")
                nc.vector.reciprocal(rec4[:], ps2q[:, :, HD])
                nc.vector.tensor_tensor(
                    ost8[:, half * 4 : half * 4 + 4, h * HD : (h + 1) * HD],
                    ps2q[:, :, 0:HD],
                    rec4[:].unsqueeze(2).broadcast_to([P, 4, HD]),
                    ALU.mult,
                )
                key = (h01, half)
                done_half.add(key)
                # once both heads of a half are normalized, its 4 i-subtiles
                # are complete -> one merged output DMA per half (a dma_start
                # issue costs ~700ns of queue time; 2 beats 8, and splitting
                # transfers across queues measured slightly worse)
                if (1 - h01, half) in done_half:
                    eng = nc.sync if half == 0 else nc.gpsimd
                    eng.dma_start(
                        outs[half * 512 : (half + 1) * 512, :].rearrange(
                            "(s p) c -> p s c", s=4
                        ),
                        ost[:, half * 4 * D : (half + 1) * 4 * D].rearrange(
                            "p (s c) -> p s c", s=4
                        ),
                    )

    _split_waits(nc)
    nc.finalize()
    return nc


_NC_CACHE = None


def _get_nc():
    global _NC_CACHE
    if _NC_CACHE is None:
        _NC_CACHE = build_nc()
    return _NC_CACHE


def make_in_maps(x, adj, W, a_src, a_dst):
    x = np.ascontiguousarray(x, dtype=np.float32)
    W = np.ascontiguousarray(W, dtype=np.float32)
    a_src = np.ascontiguousarray(a_src, dtype=np.float32)
    a_dst = np.ascontiguousarray(a_dst, dtype=np.float32)

    A_src = np.zeros((D, H), np.float32)
    A_dst = np.zeros((D, H), np.float32)
    for h in range(H):
        A_src[h * HD : (h + 1) * HD, h] = a_src[h]
        A_dst[h * HD : (h + 1) * HD, h] = a_dst[h]
    Wt = W.T.astype(np.float32)
    wd = Wt @ A_dst
    wta = np.concatenate(
        [Wt, wd, NEG_SLOPE * wd, Wt @ A_src], axis=1
    ).astype(ml_dtypes.bfloat16)

    selm = np.zeros((H, H * P), ml_dtypes.bfloat16)
    for h in range(H):
        selm[h, h * P : (h + 1) * P] = 1.0
    in_maps = []
    adjT_cache = {}
    for c in range(NCORES):
        b, ihalf = c // 2, c % 2
        ilo = ihalf * NI
        if b not in adjT_cache:
            adjT_cache[b] = adj[b].astype(ml_dtypes.bfloat16).T
        in_maps.append(
            {
                "xbt": np.ascontiguousarray(x[b].T.astype(ml_dtypes.bfloat16)),
                "xit": np.ascontiguousarray(
                    x[b, ilo : ilo + NI, :].T.astype(ml_dtypes.bfloat16)
                ),
                "wta": np.ascontiguousarray(wta),
                "adjtb": np.ascontiguousarray(adjT_cache[b][:, ilo : ilo + NI]),
                "selm": selm,
            }
        )
    return in_maps


def kernel(x, adj, W, a_src, a_dst):
    in_maps = make_in_maps(x, adj, W, a_src, a_dst)
    nc = _get_nc()
    res = run_bass_kernel_spmd(nc, in_maps, list(range(NCORES)))

    out = np.empty((B, N, D), np.float32)
    for c in range(NCORES):
        b, ihalf = c // 2, c % 2
        ilo = ihalf * NI
        out[b, ilo : ilo + NI, :] = res.results[c]["outs"]
    return out


# revision 48
# speedup vs baseline: 1.0144x; 1.0144x over previous
"""GAT layer (B=4, N=2048, D=256, H=4) on 8 trn2 NeuronCores.

Sharding: core c -> (b = c//2, i-half = c%2).  Each core computes
out[b, ihalf*1024:(ihalf+1)*1024, :]; h is computed on-device from the full
x[b] (passed pre-transposed as x[b].T, bf16).

Math: with z = s_src[i] + s_dst[j], the reference computes
    alpha = softmax_j(mask(leaky_relu(z)));  out = alpha @ h_head.
Softmax is shift-invariant per destination row i, so we use shifted
unnormalized weights (same alpha):
    P[i,j] = adj[i,j] * exp(leaky_relu(z) - s_src[i])
           = adj[i,j] * max(F1[j], E2[i] * J[j])
with F1 = exp(s_dst), J = exp(0.2*s_dst), E2 = exp(-0.8*s_src)
(z>=0 branch gives exp(s_dst)=F1; z<0 branch gives exp(0.2z-s_src)=E2*J).
Row sums come from an appended ones-column in the aggregation matmul
stationary:  psoT = [h_head | 1].T @ P^T  ->  out = psoT[:64] / psoT[64].

v3 schedule (HW-measured, 105.9us -> ~92.7us): the DVE elementwise stream
(per jt-pair-iter: 2x tensor_scalar @535 + 1x 2-head tensor_tensor @1233,
~2.1us x 32) is the pace-setter; everything else must hide under it.
  - startup (first TS 22us -> ~16us): merged single-issue DMAs (each
    dma_start costs ~650ns of queue-engine time), the three critical loads
    (xit kt0 / xit kt1 / wta) on three different queues (sync/gpsimd/
    scalar), pss->er4->e2rep emitted before h_aug, jf0/jf1 via tiny 8-col
    matmuls instead of the full h_aug, and 9x 512-col dummy matmuls to
    flip the HAM clock gate (1.2->2.4GHz) while the DMAs land.
  - tail (14us -> ~6us): pair-1 epilogues batch soT copies (ACT+DVE in
    parallel), stream the 16 fp32 transposes back-to-back on PE, normalize
    on DVE (idle in the tail), one merged output DMA per i-half.
  - hplus ones-columns via a tiny strided DVE memset (the 3.6us full-tile
    gpsimd memset blocked the gpsimd DMA queue).

MEASUREMENT TRAP: the chip has a bistable P0 power state that downclocks
ALL engines x5/6 for minutes under sustained load (exec ~93us <-> ~112us
for the SAME binary; TS 535<->665, TT 1233<->1476, MM 379<->454).  Check
per-op durations for a uniform x1.2 scale before attributing a regression
to a code change; ~4min idle restores fast mode.

Measured dead ends (some possibly confounded by P0 -- re-test carefully):
  - changing the adjt pool to 8x[128,2048] paired tiles coincided with an
    ~18% DVE slowdown (kept original pool shapes to be safe).
  - merged 4D tensor_tensor (2jt x 2head): 2751ns vs 2x1233 split (5D AP
    sub-dim overhead) AND triggers the layout slowdown above.
  - J-fold into hplus (single-op max TS, 479 vs 535ns): the per-head ACT
    scale-copies push ACT to ~55us busy and the DVE stalls ~10us on jf.
  - gpsimd tensor ops ~15us per [128,1024] + starve DVE read ports;
    scalar_tensor_tensor runs at 1x (1276ns) so fusing TS+TT loses.
  - fp8 aggregation fails accuracy (~2x the 2e-2 budget); 1024-col
    matmuls rejected by walrus (psum bank = 512 fp32); AF.Reciprocal
    blocked by bass (accuracy); 4-way split output DMA slightly worse.
"""

import sys

for _p in ("/opt/trn_rl_repo", "/root/.axon_site/_ro/trn_rl_repo"):
    if _p not in sys.path:
        sys.path.insert(0, _p)

import ml_dtypes
import numpy as np

import concourse.bass as bass
import concourse.mybir as mybir
from concourse import masks, tile
from concourse.bass_utils import run_bass_kernel_spmd
from concourse.vector_clock import ScopedClock

F32 = mybir.dt.float32
BF16 = mybir.dt.bfloat16
AF = mybir.ActivationFunctionType
ALU = mybir.AluOpType

B, N, D, H, HD = 4, 2048, 256, 4, 64
NEG_SLOPE = 0.2
P = 128
NI = N // 2          # i-rows per core (1024)
NT = N // P          # 16 j/n tiles
KT = D // P          # 2 k tiles
JT = NT
JQ = JT // 2         # 8 jt-pairs
ISUB = NI // P       # 8
NCORES = 8
WC = D + 3 * H       # 268 aug cols: [W.T | WtAdst | 0.2*WtAdst | WtAsrc]
HP1 = HD + 1         # 65
HPW = H * HP1        # 260 hplus cols per j-tile


def _patch_tile_drain():
    """walrus rejects >1 sem wait on one instruction in this toolchain; split
    the TileContext tail drain's waits across consecutive SP drains."""
    if getattr(tile.TileContext, "_drain_patched", False):
        return

    def _drain_and_barrier(self, tick_clock, wait_clock):
        nc = self.nc
        drain_inst = nc.sync.drain()
        wait_clock.add_sem_waits(
            drain_inst.ins, ScopedClock({None: tick_clock.global_clock})
        )
        si = drain_inst.ins.sync_info
        waits = list(si.on_wait) if (si and si.on_wait) else []
        if len(waits) > 1:
            ups = list(si.on_update) if (si and si.on_update) else []
            drain_inst.ins.sync_info = mybir.SyncInfo(on_wait=waits[:1], on_update=ups)
            for i in range(1, len(waits)):
                extra = nc.sync.drain()
                extra.ins.sync_info = mybir.SyncInfo(
                    on_wait=waits[i : i + 1], on_update=[]
                )
        nc.all_engine_barrier()
        assert self.sems is not None
        popped = nc._tile_sem_poison_stack.pop()
        assert popped is self._sem_poison
        nc.clear_and_free_semaphores(list(self.sems.allocated().values()))
        nc.all_engine_barrier()

    tile.TileContext._drain_and_barrier = _drain_and_barrier
    tile.TileContext._drain_patched = True


def _split_waits(nc, maxw=1):
    """Hoist excess sem waits onto same-engine EventSemaphore carriers placed
    just before the instruction (same engine + program order => equivalent)."""
    n_split = 0
    for f in nc.m.functions:
        for bb in f.blocks:
            insts = list(bb.instructions)
            out = []
            changed = False
            for inst in insts:
                si = inst.sync_info
                waits = list(si.on_wait) if (si and si.on_wait) else []
                if len(waits) > maxw and inst.engine is not None:
                    changed = True
                    extra, keep = waits[:-maxw], waits[-maxw:]
                    for k in range(0, len(extra), maxw):
                        d = mybir.InstEventSemaphore(
                            name=f"{inst.name}-wsplit{k}", ins=[], outs=[]
                        )
                        d.engine = inst.engine
                        d.sync_info = mybir.SyncInfo(
                            on_wait=extra[k : k + maxw], on_update=[]
                        )
                        out.append(d)
                        n_split += 1
                    ups = list(si.on_update) if (si and si.on_update) else []
                    inst.sync_info = mybir.SyncInfo(on_wait=keep, on_update=ups)
                out.append(inst)
            if changed:
                bb.instructions = out
    return n_split


def build_nc():
    _patch_tile_drain()
    nc = bass.Bass("TRN2", target_bir_lowering=False, debug=False)

    xbt = nc.dram_tensor("xbt", [D, N], BF16, kind="ExternalInput")    # x[b].T (cols rotated so own i-half first)
    wta = nc.dram_tensor("wta", [D, WC], BF16, kind="ExternalInput")
    adjtb = nc.dram_tensor("adjtb", [N, NI], BF16, kind="ExternalInput")
    selm = nc.dram_tensor("selm", [H, H * P], BF16, kind="ExternalInput")
    outs = nc.dram_tensor("outs", [NI, D], F32, kind="ExternalOutput")

    with tile.TileContext(nc) as tc:
        with (
            tc.tile_pool(name="const", bufs=1) as constp,
            tc.tile_pool(name="big", bufs=1) as bigp,
            tc.tile_pool(name="rows", bufs=1) as rowsp,
            tc.tile_pool(name="jf", bufs=16) as jfp,
            tc.tile_pool(name="adjt", bufs=17) as adjtp,
            tc.tile_pool(name="vwork", bufs=3) as vp,
            tc.tile_pool(name="ptwork", bufs=5) as ptp,
            tc.tile_pool(name="sot", bufs=5) as sotp,
            tc.tile_pool(name="small", bufs=6) as smallp,
            tc.tile_pool(name="psall", bufs=1, space="PSUM") as psall,
        ):
            # transient psum tiles round-robin banks 4-7; psoT/ps2 use banks 0-3
            ps_ctr = [0]

            def ps_tile(shape, name, tag=None):
                if tag is None:
                    tag = f"bank{4 + ps_ctr[0] % 4}"
                    ps_ctr[0] += 1
                return psall.tile(shape, F32, tag=tag, name=name)

            pe_prev = [None]

            def pe(bi):
                # pin PE stream order: PSUM accumulation groups must stay
                # contiguous on PE (interleaving corrupts accumulation on HW)
                if pe_prev[0] is not None:
                    tile.add_dep_helper(bi.ins, pe_prev[0], reason="pe-order")
                pe_prev[0] = bi.ins
                return bi

            ident = constp.tile([P, P], F32, tag="ident")
            masks.make_identity(nc, ident[:])

            wta_all = constp.tile([P, KT * WC], BF16, tag="wta01", name="wta_all")

            def wta_mov(kt):
                return wta_all[:, kt * WC : (kt + 1) * WC]

            def wta_srcc(kt):
                return wta_all[:, kt * WC + D + 2 * H : kt * WC + D + 3 * H]
            sel_sb = constp.tile([H, H * P], BF16, tag="selm")
            sels = [sel_sb[:, h * P : (h + 1) * P] for h in range(H)]

            xt_sb = bigp.tile([P, KT * N], BF16, tag="xt")
            # only the ones-columns of hplus need init (the h_head parts are
            # fully written by the ACT copies); a tiny strided DVE memset
            # keeps the 3.6us full-tile gpsimd memset off the DMA queue.
            # NOTE: folding J into hplus (single-op max TS) was tried and
            # REGRESSED +9us: the per-head ACT scale-copies push ACT to
            # ~55us busy and the DVE stalls waiting on jf tiles.
            hplus = bigp.tile([P, NT * HPW], BF16, tag="hplus")
            hp4i = hplus[:].rearrange("p (t h c) -> p t h c", t=NT, h=H)
            nc.vector.memset(hp4i[:, :, :, HD : HD + 1], 1.0)
            adjts = [
                adjtp.tile([P, NI], BF16, tag="adjt", name=f"adjt_{jt}")
                for jt in range(JT)
            ]

            # ---- DMA schedule: transfers on one queue serialize, so the two
            # queues each carry one kt-half of the critical tensors.  Order:
            # wta (small, needed by every matmul) then xit (pss chain) then
            # the first 256 xt cols (jf0/jf1 + h_aug nt0/nt1) then adj jt0/1
            # (first TT), then the bulk.
            # first 0:NI columns of each kt-half (own i-half, rotated to the
            # front) on the two queues: these feed pss, e2rep, jf0-7 and
            # h_aug nt0-7; wta on scalar (ACT idle until the er4 exps).
            # NOTE: routing bulk tiles through the scalar/ACT queue measured
            # a +19us regression -- only wta goes there.
            nc.sync.dma_start(xt_sb[:, 0:NI], xbt[0:P, 0:NI])
            nc.gpsimd.dma_start(xt_sb[:, N : N + NI], xbt[P : 2 * P, 0:NI])
            nc.scalar.dma_start(
                wta_all[:].rearrange("p (k c) -> p k c", k=KT),
                wta[:, :].rearrange("(k p) c -> p k c", k=KT),
            )
            nc.sync.dma_start(sel_sb[:], selm[:])
            for jt in range(2):
                nc.gpsimd.dma_start(adjts[jt][:], adjtb[jt * P : (jt + 1) * P, :])
            # interleave the remaining xt half with the adj tiles so adj
            # jt2-8 land before the DVE consumes them (~2.05us per jt)
            nc.sync.dma_start(
                xt_sb[:, NI : NI + 512], xbt[0:P, NI : NI + 512])
            for jt in range(2, 6):
                eng = nc.gpsimd if jt % 2 == 0 else nc.sync
                eng.dma_start(adjts[jt][:], adjtb[jt * P : (jt + 1) * P, :])
            nc.gpsimd.dma_start(
                xt_sb[:, N + NI : N + NI + 512], xbt[P : 2 * P, NI : NI + 512])
            nc.sync.dma_start(
                xt_sb[:, NI + 512 : N], xbt[0:P, NI + 512 : N])
            for jt in range(6, 10):
                eng = nc.gpsimd if jt % 2 == 0 else nc.sync
                eng.dma_start(adjts[jt][:], adjtb[jt * P : (jt + 1) * P, :])
            nc.gpsimd.dma_start(
                xt_sb[:, N + NI + 512 : 2 * N], xbt[P : 2 * P, NI + 512 : N])
            for jt in range(10, JT):
                eng = nc.gpsimd if jt % 2 == 0 else nc.sync
                eng.dma_start(adjts[jt][:], adjtb[jt * P : (jt + 1) * P, :])

            # hplus memset gates the per-nt strided copies; emitted before any
            # gpsimd DMA issues so it lands at t~0, not behind the DMA queue
            hp4 = hplus[:].rearrange("p (t h c) -> p t h c", t=NT, h=H)

            # HAM warmup: PE is otherwise idle until the xit DMA lands, so
            # the startup matmul chain runs at the cold 1.2GHz clock.  ~3.5us
            # of dummy matmul activity flips the HAM gate to 2.4GHz first.
            # hplus is garbage here; pss start=True clears the psum after.
            # 512-col moving so each dummy occupies the array ~427ns; 9 of
            # them span ~3.8us = a full HAM window, ending as the xit DMA
            # lands (~11.4us) so the real chain runs at 2.4GHz
            psd = ps_tile([8, 512], "psd", tag="bank4")
            for _ in range(9):
                pe(nc.tensor.matmul(
                    psd[:], hplus[0:P, 0:8], hplus[:, 0:512],
                    start=True, stop=True,
                ))

            # ---- s_srcT (all heads) -> E2 rows [4, NI] (bf16) ----
            er4 = rowsp.tile([H, NI], BF16, tag="er4")
            for c in range(NI // 512):
                pss = ps_tile([H, 512], f"pss_{c}")
                for kt in range(KT):
                    pe(nc.tensor.matmul(
                        pss[:],
                        wta_srcc(kt),
                        xt_sb[:, kt * N + c * 512 : kt * N + (c + 1) * 512],
                        start=(kt == 0),
                        stop=(kt == KT - 1),
                    ))
                nc.scalar.activation(
                    er4[:, c * 512 : (c + 1) * 512],
                    pss[:],
                    AF.Exp,
                    scale=-(1.0 - NEG_SLOPE),
                )
            e2rep = bigp.tile([P, H * NI], BF16, tag="e2rep")

            def emit_e2rep(h):
                for c in range(NI // 512):
                    psb = ps_tile([P, 512], f"psb_{h}_{c}")
                    pe(nc.tensor.matmul(
                        psb[:], sels[h], er4[0:H, c * 512 : (c + 1) * 512]
                    ))
                    nc.scalar.activation(
                        e2rep[:, h * NI + c * 512 : h * NI + (c + 1) * 512],
                        psb[:],
                        AF.Copy,
                    )

            # ---- h_aug = x @ wta (bf16); JF = [F1|J]; hplus strided copy ----
            jf_tiles = {}

            def emit_haug(nt):
                psh = ps_tile([P, WC], f"psh_{nt}")
                for kt in range(KT):
                    pe(nc.tensor.matmul(
                        psh[:],
                        xt_sb[:, kt * N + nt * P : kt * N + (nt + 1) * P],
                        wta_mov(kt),
                        start=(kt == 0),
                        stop=(kt == KT - 1),
                    ))
                if nt not in jf_tiles:
                    jf = jfp.tile([P, 2 * H], F32, tag="jf", name=f"jf_{nt}")
                    nc.scalar.activation(jf[:], psh[:, D : D + 2 * H], AF.Exp)
                    jf_tiles[nt] = jf
                nc.scalar.activation(
                    hp4[:, nt, :, 0:HD],
                    psh[:, 0:D].rearrange("p (h c) -> p h c", h=H),
                    AF.Copy,
                )

            def emit_jf_early(nt):
                # jf only needs the 2H score columns of psh -- a ~190ns tiny
                # matmul per kt instead of waiting on the full h_aug
                psj = ps_tile([P, 2 * H], f"psj_{nt}")
                for kt in range(KT):
                    pe(nc.tensor.matmul(
                        psj[:],
                        xt_sb[:, kt * N + nt * P : kt * N + (nt + 1) * P],
                        wta_all[:, kt * WC + D : kt * WC + D + 2 * H],
                        start=(kt == 0),
                        stop=(kt == KT - 1),
                    ))
                jf = jfp.tile([P, 2 * H], F32, tag="jf", name=f"jf_{nt}")
                nc.scalar.activation(jf[:], psj[:], AF.Exp)
                jf_tiles[nt] = jf

            # critical order for the first TS/TT of pair 0 (heads 0/1, jt 0/1):
            # pss -> e2rep h0/h1 (copies on ACT, keeping the DVE queue clear
            # for the first TS); jf0/jf1 via tiny early matmuls; the full
            # h_aug (for hplus) follows
            emit_e2rep(0)
            emit_jf_early(0)
            emit_jf_early(1)
            emit_e2rep(1)
            # pre-produce jf for nt 2..7 too: the tiny matmuls are ~2x100ns
            # on a warm PE, and pulling the exps ahead of the bulky hp/e2rep
            # ACT ops keeps the DVE from stalling on jf supply early in the
            # main loop (measured ~4us of stretched iterations there)
            for nt in range(2, 8):
                emit_jf_early(nt)
            emit_haug(0)
            emit_haug(1)
            emit_haug(2)
            emit_haug(3)
            emit_e2rep(2)
            emit_e2rep(3)
            for nt in range(4, NT):
                emit_haug(nt)

            # ---- main: P^T construction (DVE) + aggregation + epilogue ----
            ost = bigp.tile([P, ISUB * D], F32, tag="ost")
            ost8 = ost[:].rearrange("p (s c) -> p s c", s=ISUB)

            # epilogue for one (h, half) combo (pair-0 / mid-kernel style:
            # ACT-heavy, stays off the DVE critical path)
            def emit_epilogue(pair, h01, half, psoT):
                h = 2 * pair + h01
                soT = sotp.tile([HP1, 512], F32, tag="soT", name=f"soT_{h}_{half}")
                nc.scalar.activation(soT[:], psoT[:], AF.Copy)
                ps2 = psall.tile(
                    [P, H * HP1], F32, tag=f"bank{h01 * 2 + half}",
                    name=f"ps2_{h}_{half}",
                )
                for q in range(4):
                    pe(nc.tensor.transpose(
                        ps2[:, q * HP1 : (q + 1) * HP1],
                        soT[:, q * P : (q + 1) * P],
                        ident[0:HP1, 0:HP1],
                    ))
                ps2q = ps2[:].rearrange("p (q c) -> p q c", q=4)
                rec4 = smallp.tile([P, 4], F32, tag="rec", name=f"rec_{h}_{half}")
                nc.vector.reciprocal(rec4[:], ps2q[:, :, HD])
                for q in range(4):
                    nc.scalar.activation(
                        ost8[:, half * 4 + q, h * HD : (h + 1) * HD],
                        ps2q[:, q, 0:HD],
                        AF.Copy,
                        scale=rec4[:, q : q + 1],
                    )

            pending_epi = []
            for pair in range(2):
                # pair0 accumulates on banks 0-3, pair1 on banks 4-7 (the
                # h_aug/pss transients are done by then): decoupling the
                # banks lets pair0's epilogues (ps2 on banks 0-3) spread
                # across pair1's first iterations instead of wedging between
                # the pairs, where the DVE reciprocals stalled the stream.
                while pending_epi and pair == 0:
                    emit_epilogue(*pending_epi.pop(0))
                psoTs = {}
                for h01 in range(2):
                    for half in range(2):
                        psoTs[(h01, half)] = psall.tile(
                            [HP1, 512], F32,
                            tag=f"bank{4 * pair + h01 * 2 + half}",
                            name=f"psoT_{2 * pair + h01}_{half}",
                        )
                for jt in range(JT):
                    if pending_epi and jt in (2, 4, 6, 8):
                        emit_epilogue(*pending_epi.pop(0))
                    adjv = adjts[jt][:]
                    # v = max(e2rep*J, F1) per head; pt = v*adj (merged 2-head)
                    v2 = vp.tile([P, 2 * NI], BF16, tag="v", name=f"v_{pair}_{jt}")
                    for h01 in range(2):
                        h = 2 * pair + h01
                        nc.vector.tensor_scalar(
                            v2[:, h01 * NI : (h01 + 1) * NI],
                            e2rep[:, h * NI : (h + 1) * NI],
                            jf_tiles[jt][:, H + h : H + h + 1],
                            jf_tiles[jt][:, h : h + 1],
                            ALU.mult,
                            ALU.max,
                        )
                    pt2 = ptp.tile([P, 2 * NI], BF16, tag="pt", name=f"pt_{pair}_{jt}")
                    nc.vector.tensor_tensor(
                        pt2[:].rearrange("p (g c) -> p g c", g=2),
                        v2[:].rearrange("p (g c) -> p g c", g=2),
                        adjv.unsqueeze(1).broadcast_to([P, 2, NI]),
                        ALU.mult,
                    )
                    # jt-major aggregation: 4 interleaved accumulation groups
                    # (verified on HW: per-cell has_written bits make
                    # interleaved groups on different banks safe)
                    for h01 in range(2):
                        h = 2 * pair + h01
                        for half in range(2):
                            pe(nc.tensor.matmul(
                                psoTs[(h01, half)][:],
                                hplus[:, jt * HPW + h * HP1 : jt * HPW + (h + 1) * HP1],
                                pt2[:, h01 * NI + half * 512 : h01 * NI + (half + 1) * 512],
                                start=(jt == 0),
                                stop=(jt == JT - 1),
                                skip_group_check=True,
                            ))
                for half in range(2):
                    for h01 in range(2):
                        pending_epi.append((pair, h01, half, psoTs[(h01, half)]))

            # ---- tail: pair-1 epilogues, restructured for minimum latency:
            # all soT copies first (split ACT/DVE so they run in parallel),
            # then the 16 transposes stream back-to-back on PE, then DVE
            # reciprocal + DVE normalize (DVE is idle in the tail), with the
            # output DMAs interleaved per half.
            tail = [pending_epi.pop(0) for _ in range(4)]
            soTs = {}
            for k, (pair, h01, half, psoT) in enumerate(tail):
                h = 2 * pair + h01
                soT = sotp.tile(
                    [HP1, 512], F32, tag="soT", name=f"soTt_{h}_{half}"
                )
                if k % 2 == 0:
                    nc.scalar.activation(soT[:], psoT[:], AF.Copy)
                else:
                    nc.vector.tensor_copy(soT[:], psoT[:])
                soTs[(h01, half)] = soT
            ps2s = {}
            for pair, h01, half, psoT in tail:
                h = 2 * pair + h01
                ps2 = psall.tile(
                    [P, H * HP1], F32, tag=f"bank{h01 * 2 + half}",
                    name=f"ps2t_{h}_{half}",
                )
                for q in range(4):
                    pe(nc.tensor.transpose(
                        ps2[:, q * HP1 : (q + 1) * HP1],
                        soTs[(h01, half)][:, q * P : (q + 1) * P],
                        ident[0:HP1, 0:HP1],
                    ))
                ps2s[(h01, half)] = ps2[:].rearrange("p (q c) -> p q c", q=4)
            done_half = set()
            for pair, h01, half, psoT in tail:
                h = 2 * pair + h01
                ps2q = ps2s[(h01, half)]
                rec4 = smallp.tile([P, 4], F32, tag="rec", name=f"rect_{h}_{half}")
                nc.vector.reciprocal(rec4[:], ps2q[:, :, HD])
                nc.vector.tensor_tensor(
                    ost8[:, half * 4 : half * 4 + 4, h * HD : (h + 1) * HD],
                    ps2q[:, :, 0:HD],
                    rec4[:].unsqueeze(2).broadcast_to([P, 4, HD]),
                    ALU.mult,
                )
                key = (h01, half)
                done_half.add(key)
                # once both heads of a half are normalized, its 4 i-subtiles
                # are complete -> one merged output DMA per half (a dma_start
                # issue costs ~700ns of queue time; 2 beats 8, and splitting
                # transfers across queues measured slightly worse)
                if (1 - h01, half) in done_half:
                    eng = nc.sync if half == 0 else nc.gpsimd
                    eng.dma_start(
                        outs[half * 512 : (half + 1) * 512, :].rearrange(
                            "(s p) c -> p s c", s=4
                        ),
                        ost[:, half * 4 * D : (half + 1) * 4 * D].rearrange(
                            "p (s c) -> p s c", s=4
                        ),
                    )

    _split_waits(nc)
    nc.finalize()
    return nc


_NC_CACHE = None


def _get_nc():
    global _NC_CACHE
    if _NC_CACHE is None:
        _NC_CACHE = build_nc()
    return _NC_CACHE


def make_in_maps(x, adj, W, a_src, a_dst):
    x = np.ascontiguousarray(x, dtype=np.float32)
    W = np.ascontiguousarray(W, dtype=np.float32)
    a_src = np.ascontiguousarray(a_src, dtype=np.float32)
    a_dst = np.ascontiguousarray(a_dst, dtype=np.float32)

    A_src = np.zeros((D, H), np.float32)
    A_dst = np.zeros((D, H), np.float32)
    for h in range(H):
        A_src[h * HD : (h + 1) * HD, h] = a_src[h]
        A_dst[h * HD : (h + 1) * HD, h] = a_dst[h]
    Wt = W.T.astype(np.float32)
    wd = Wt @ A_dst
    wta = np.concatenate(
        [Wt, wd, NEG_SLOPE * wd, Wt @ A_src], axis=1
    ).astype(ml_dtypes.bfloat16)

    selm = np.zeros((H, H * P), ml_dtypes.bfloat16)
    for h in range(H):
        selm[h, h * P : (h + 1) * P] = 1.0
    in_maps = []
    adjT_cache = {}
    for c in range(NCORES):
        b, ihalf = c // 2, c % 2
        ilo = ihalf * NI
        if b not in adjT_cache:
            adjT_cache[b] = adj[b].astype(ml_dtypes.bfloat16).T
        # rotate x columns (and adj rows to match) so this core's i-half is
        # always xbt cols 0:NI -- the first xt chunk then feeds pss, jf0-7
        # and h_aug at once, and the duplicate 0.5MB xit load disappears
        xbt_c = np.roll(x[b].T.astype(ml_dtypes.bfloat16), -ilo, axis=1)
        adjt_c = np.roll(adjT_cache[b][:, ilo : ilo + NI], -ilo, axis=0)
        in_maps.append(
            {
                "xbt": np.ascontiguousarray(xbt_c),
                "wta": np.ascontiguousarray(wta),
                "adjtb": np.ascontiguousarray(adjt_c),
                "selm": selm,
            }
        )
    return in_maps


def kernel(x, adj, W, a_src, a_dst):
    in_maps = make_in_maps(x, adj, W, a_src, a_dst)
    nc = _get_nc()
    res = run_bass_kernel_spmd(nc, in_maps, list(range(NCORES)))

    out = np.empty((B, N, D), np.float32)
    for c in range(NCORES):
        b, ihalf = c // 2, c % 2
        ilo = ihalf * NI
        out[b, ilo : ilo + NI, :] = res.results[c]["outs"]
    return out


# revision 50
# speedup vs baseline: 1.0203x; 1.0058x over previous
"""GAT layer (B=4, N=2048, D=256, H=4) on 8 trn2 NeuronCores.

Sharding: core c -> (b = c//2, i-half = c%2).  Each core computes
out[b, ihalf*1024:(ihalf+1)*1024, :]; h is computed on-device from the full
x[b] (passed pre-transposed as x[b].T, bf16).

Math: with z = s_src[i] + s_dst[j], the reference computes
    alpha = softmax_j(mask(leaky_relu(z)));  out = alpha @ h_head.
Softmax is shift-invariant per destination row i, so we use shifted
unnormalized weights (same alpha):
    P[i,j] = adj[i,j] * exp(leaky_relu(z) - s_src[i])
           = adj[i,j] * max(F1[j], E2[i] * J[j])
with F1 = exp(s_dst), J = exp(0.2*s_dst), E2 = exp(-0.8*s_src)
(z>=0 branch gives exp(s_dst)=F1; z<0 branch gives exp(0.2z-s_src)=E2*J).
Row sums come from an appended ones-column in the aggregation matmul
stationary:  psoT = [h_head | 1].T @ P^T  ->  out = psoT[:64] / psoT[64].

v4 schedule (HW-measured, 105.9us -> ~92.7-93.2us): the DVE elementwise
stream (per jt-pair-iter: 2x tensor_scalar @535 + 1x 2-head tensor_tensor
@1233, ~2.05us x 32) is the pace-setter; everything else must hide under
it.
  - x is passed per-core with COLUMNS ROTATED so the core's own i-half is
    always xbt[:, 0:NI] (adj rows rotated to match; the j-sum is order-
    invariant): the first xt chunk feeds pss, e2rep, jf0-7 AND h_aug at
    once and the duplicate 0.5MB xit load is gone, relieving the DMA
    window (4MB of adj + 1MB x on ~170GB/s across sync+gpsimd queues is
    the binding constraint from ~8-30us).
  - startup (first TS 22us -> ~16us): merged single-issue DMAs (each
    dma_start costs ~650ns of queue-engine time), kt-halves of xt on the
    two DMA queues with wta on scalar, pss->er4->e2rep emitted before
    h_aug, jf for nt0-7 via tiny 8-col matmuls ahead of the bulky ACT
    copies (DVE stalls on jf supply otherwise), and 9x 512-col dummy
    matmuls to flip the HAM clock gate (1.2->2.4GHz) while DMAs land.
  - pair0 accumulates on psum banks 0-3, pair1 on banks 4-7, so pair0's
    epilogues spread over pair1's first iterations instead of wedging
    their DVE reciprocals between the pairs.
  - tail (14us -> ~6us): pair-1 epilogues batch soT copies (ACT+DVE in
    parallel), stream the 16 fp32 transposes back-to-back on PE, normalize
    on DVE (idle in the tail), one merged output DMA per i-half.
  - hplus ones-columns via a tiny strided DVE memset (the 3.6us full-tile
    gpsimd memset blocked the gpsimd DMA queue).

MEASUREMENT TRAP: the chip has a bistable P0 power state that downclocks
ALL engines x5/6 for minutes under sustained load (exec ~93us <-> ~112us
for the SAME binary; TS 535<->665, TT 1233<->1476, MM 379<->454).  Check
per-op durations for a uniform x1.2 scale before attributing a regression
to a code change; ~4min idle restores fast mode.

Measured dead ends (some possibly confounded by P0 -- re-test carefully):
  - changing the adjt pool to 8x[128,2048] paired tiles coincided with an
    ~18% DVE slowdown (kept original pool shapes to be safe).
  - merged 4D tensor_tensor (2jt x 2head): 2751ns vs 2x1233 split (5D AP
    sub-dim overhead) AND triggers the layout slowdown above.
  - J-fold into hplus (single-op max TS, 479 vs 535ns): the per-head ACT
    scale-copies push ACT to ~55us busy and the DVE stalls ~10us on jf.
  - gpsimd tensor ops ~15us per [128,1024] + starve DVE read ports;
    scalar_tensor_tensor runs at 1x (1276ns) so fusing TS+TT loses.
  - fp8 aggregation fails accuracy (~2x the 2e-2 budget); 1024-col
    matmuls rejected by walrus (psum bank = 512 fp32); AF.Reciprocal
    blocked by bass (accuracy); 4-way split output DMA slightly worse.
"""

import sys

for _p in ("/opt/trn_rl_repo", "/root/.axon_site/_ro/trn_rl_repo"):
    if _p not in sys.path:
        sys.path.insert(0, _p)

import ml_dtypes
import numpy as np

import concourse.bass as bass
import concourse.mybir as mybir
from concourse import masks, tile
from concourse.bass_utils import run_bass_kernel_spmd
from concourse.vector_clock import ScopedClock

F32 = mybir.dt.float32
BF16 = mybir.dt.bfloat16
AF = mybir.ActivationFunctionType
ALU = mybir.AluOpType

B, N, D, H, HD = 4, 2048, 256, 4, 64
NEG_SLOPE = 0.2
P = 128
NI = N // 2          # i-rows per core (1024)
NT = N // P          # 16 j/n tiles
KT = D // P          # 2 k tiles
JT = NT
JQ = JT // 2         # 8 jt-pairs
ISUB = NI // P       # 8
NCORES = 8
WC = D + 3 * H       # 268 aug cols: [W.T | WtAdst | 0.2*WtAdst | WtAsrc]
HP1 = HD + 1         # 65
HPW = H * HP1        # 260 hplus cols per j-tile


def _patch_tile_drain():
    """walrus rejects >1 sem wait on one instruction in this toolchain; split
    the TileContext tail drain's waits across consecutive SP drains."""
    if getattr(tile.TileContext, "_drain_patched", False):
        return

    def _drain_and_barrier(self, tick_clock, wait_clock):
        nc = self.nc
        drain_inst = nc.sync.drain()
        wait_clock.add_sem_waits(
            drain_inst.ins, ScopedClock({None: tick_clock.global_clock})
        )
        si = drain_inst.ins.sync_info
        waits = list(si.on_wait) if (si and si.on_wait) else []
        if len(waits) > 1:
            ups = list(si.on_update) if (si and si.on_update) else []
            drain_inst.ins.sync_info = mybir.SyncInfo(on_wait=waits[:1], on_update=ups)
            for i in range(1, len(waits)):
                extra = nc.sync.drain()
                extra.ins.sync_info = mybir.SyncInfo(
                    on_wait=waits[i : i + 1], on_update=[]
                )
        nc.all_engine_barrier()
        assert self.sems is not None
        popped = nc._tile_sem_poison_stack.pop()
        assert popped is self._sem_poison
        nc.clear_and_free_semaphores(list(self.sems.allocated().values()))
        nc.all_engine_barrier()

    tile.TileContext._drain_and_barrier = _drain_and_barrier
    tile.TileContext._drain_patched = True


def _split_waits(nc, maxw=1):
    """Hoist excess sem waits onto same-engine EventSemaphore carriers placed
    just before the instruction (same engine + program order => equivalent)."""
    n_split = 0
    for f in nc.m.functions:
        for bb in f.blocks:
            insts = list(bb.instructions)
            out = []
            changed = False
            for inst in insts:
                si = inst.sync_info
                waits = list(si.on_wait) if (si and si.on_wait) else []
                if len(waits) > maxw and inst.engine is not None:
                    changed = True
                    extra, keep = waits[:-maxw], waits[-maxw:]
                    for k in range(0, len(extra), maxw):
                        d = mybir.InstEventSemaphore(
                            name=f"{inst.name}-wsplit{k}", ins=[], outs=[]
                        )
                        d.engine = inst.engine
                        d.sync_info = mybir.SyncInfo(
                            on_wait=extra[k : k + maxw], on_update=[]
                        )
                        out.append(d)
                        n_split += 1
                    ups = list(si.on_update) if (si and si.on_update) else []
                    inst.sync_info = mybir.SyncInfo(on_wait=keep, on_update=ups)
                out.append(inst)
            if changed:
                bb.instructions = out
    return n_split


def build_nc():
    _patch_tile_drain()
    nc = bass.Bass("TRN2", target_bir_lowering=False, debug=False)

    xbt = nc.dram_tensor("xbt", [D, N], BF16, kind="ExternalInput")    # x[b].T (cols rotated so own i-half first)
    wta = nc.dram_tensor("wta", [D, WC], BF16, kind="ExternalInput")
    adjtb = nc.dram_tensor("adjtb", [N, NI], BF16, kind="ExternalInput")
    selm = nc.dram_tensor("selm", [H, H * P], BF16, kind="ExternalInput")
    outs = nc.dram_tensor("outs", [NI, D], F32, kind="ExternalOutput")

    with tile.TileContext(nc) as tc:
        with (
            tc.tile_pool(name="const", bufs=1) as constp,
            tc.tile_pool(name="big", bufs=1) as bigp,
            tc.tile_pool(name="rows", bufs=1) as rowsp,
            tc.tile_pool(name="jf", bufs=16) as jfp,
            tc.tile_pool(name="adjt", bufs=17) as adjtp,
            tc.tile_pool(name="vwork", bufs=3) as vp,
            # 8 pt bufs: while the h_aug backlog runs (16-35us) the PE needs
            # ~2.5us/jt vs the DVE's 2.05 -- deeper pt buffering keeps the
            # DVE from stalling on tile reuse until the PE catches up
            tc.tile_pool(name="ptwork", bufs=8) as ptp,
            tc.tile_pool(name="sot", bufs=5) as sotp,
            tc.tile_pool(name="small", bufs=6) as smallp,
            tc.tile_pool(name="psall", bufs=1, space="PSUM") as psall,
        ):
            # transient psum tiles round-robin banks 4-7; psoT/ps2 use banks 0-3
            ps_ctr = [0]

            def ps_tile(shape, name, tag=None):
                if tag is None:
                    tag = f"bank{4 + ps_ctr[0] % 4}"
                    ps_ctr[0] += 1
                return psall.tile(shape, F32, tag=tag, name=name)

            pe_prev = [None]

            def pe(bi):
                # pin PE stream order: PSUM accumulation groups must stay
                # contiguous on PE (interleaving corrupts accumulation on HW)
                if pe_prev[0] is not None:
                    tile.add_dep_helper(bi.ins, pe_prev[0], reason="pe-order")
                pe_prev[0] = bi.ins
                return bi

            ident = constp.tile([P, P], F32, tag="ident")
            masks.make_identity(nc, ident[:])

            wta_all = constp.tile([P, KT * WC], BF16, tag="wta01", name="wta_all")

            def wta_mov(kt):
                return wta_all[:, kt * WC : (kt + 1) * WC]

            def wta_srcc(kt):
                return wta_all[:, kt * WC + D + 2 * H : kt * WC + D + 3 * H]
            sel_sb = constp.tile([H, H * P], BF16, tag="selm")
            sels = [sel_sb[:, h * P : (h + 1) * P] for h in range(H)]

            xt_sb = bigp.tile([P, KT * N], BF16, tag="xt")
            # only the ones-columns of hplus need init (the h_head parts are
            # fully written by the ACT copies); a tiny strided DVE memset
            # keeps the 3.6us full-tile gpsimd memset off the DMA queue.
            # NOTE: folding J into hplus (single-op max TS) was tried and
            # REGRESSED +9us: the per-head ACT scale-copies push ACT to
            # ~55us busy and the DVE stalls waiting on jf tiles.
            hplus = bigp.tile([P, NT * HPW], BF16, tag="hplus")
            hp4i = hplus[:].rearrange("p (t h c) -> p t h c", t=NT, h=H)
            nc.vector.memset(hp4i[:, :, :, HD : HD + 1], 1.0)
            adjts = [
                adjtp.tile([P, NI], BF16, tag="adjt", name=f"adjt_{jt}")
                for jt in range(JT)
            ]

            # ---- DMA schedule: transfers on one queue serialize, so the two
            # queues each carry one kt-half of the critical tensors.  Order:
            # wta (small, needed by every matmul) then xit (pss chain) then
            # the first 256 xt cols (jf0/jf1 + h_aug nt0/nt1) then adj jt0/1
            # (first TT), then the bulk.
            # first 0:NI columns of each kt-half (own i-half, rotated to the
            # front) on the two queues: these feed pss, e2rep, jf0-7 and
            # h_aug nt0-7; wta on scalar (ACT idle until the er4 exps).
            # NOTE: routing bulk tiles through the scalar/ACT queue measured
            # a +19us regression -- only wta goes there.
            nc.sync.dma_start(xt_sb[:, 0:NI], xbt[0:P, 0:NI])
            nc.gpsimd.dma_start(xt_sb[:, N : N + NI], xbt[P : 2 * P, 0:NI])
            nc.scalar.dma_start(
                wta_all[:].rearrange("p (k c) -> p k c", k=KT),
                wta[:, :].rearrange("(k p) c -> p k c", k=KT),
            )
            nc.sync.dma_start(sel_sb[:], selm[:])
            for jt in range(2):
                nc.gpsimd.dma_start(adjts[jt][:], adjtb[jt * P : (jt + 1) * P, :])
            # interleave the remaining xt half with the adj tiles so adj
            # jt2-8 land before the DVE consumes them (~2.05us per jt)
            nc.sync.dma_start(
                xt_sb[:, NI : NI + 512], xbt[0:P, NI : NI + 512])
            for jt in range(2, 6):
                eng = nc.gpsimd if jt % 2 == 0 else nc.sync
                eng.dma_start(adjts[jt][:], adjtb[jt * P : (jt + 1) * P, :])
            nc.gpsimd.dma_start(
                xt_sb[:, N + NI : N + NI + 512], xbt[P : 2 * P, NI : NI + 512])
            nc.sync.dma_start(
                xt_sb[:, NI + 512 : N], xbt[0:P, NI + 512 : N])
            for jt in range(6, 10):
                eng = nc.gpsimd if jt % 2 == 0 else nc.sync
                eng.dma_start(adjts[jt][:], adjtb[jt * P : (jt + 1) * P, :])
            nc.gpsimd.dma_start(
                xt_sb[:, N + NI + 512 : 2 * N], xbt[P : 2 * P, NI + 512 : N])
            for jt in range(10, JT):
                eng = nc.gpsimd if jt % 2 == 0 else nc.sync
                eng.dma_start(adjts[jt][:], adjtb[jt * P : (jt + 1) * P, :])

            # hplus memset gates the per-nt strided copies; emitted before any
            # gpsimd DMA issues so it lands at t~0, not behind the DMA queue
            hp4 = hplus[:].rearrange("p (t h c) -> p t h c", t=NT, h=H)

            # HAM warmup: PE is otherwise idle until the xit DMA lands, so
            # the startup matmul chain runs at the cold 1.2GHz clock.  ~3.5us
            # of dummy matmul activity flips the HAM gate to 2.4GHz first.
            # hplus is garbage here; pss start=True clears the psum after.
            # 512-col moving so each dummy occupies the array ~427ns; 9 of
            # them span ~3.8us = a full HAM window, ending as the xit DMA
            # lands (~11.4us) so the real chain runs at 2.4GHz
            psd = ps_tile([8, 512], "psd", tag="bank4")
            for _ in range(9):
                pe(nc.tensor.matmul(
                    psd[:], hplus[0:P, 0:8], hplus[:, 0:512],
                    start=True, stop=True,
                ))

            # ---- s_srcT (all heads) -> E2 rows [4, NI] (bf16) ----
            er4 = rowsp.tile([H, NI], BF16, tag="er4")
            for c in range(NI // 512):
                pss = ps_tile([H, 512], f"pss_{c}")
                for kt in range(KT):
                    pe(nc.tensor.matmul(
                        pss[:],
                        wta_srcc(kt),
                        xt_sb[:, kt * N + c * 512 : kt * N + (c + 1) * 512],
                        start=(kt == 0),
                        stop=(kt == KT - 1),
                    ))
                nc.scalar.activation(
                    er4[:, c * 512 : (c + 1) * 512],
                    pss[:],
                    AF.Exp,
                    scale=-(1.0 - NEG_SLOPE),
                )
            e2rep = bigp.tile([P, H * NI], BF16, tag="e2rep")

            def emit_e2rep(h):
                for c in range(NI // 512):
                    psb = ps_tile([P, 512], f"psb_{h}_{c}")
                    pe(nc.tensor.matmul(
                        psb[:], sels[h], er4[0:H, c * 512 : (c + 1) * 512]
                    ))
                    nc.scalar.activation(
                        e2rep[:, h * NI + c * 512 : h * NI + (c + 1) * 512],
                        psb[:],
                        AF.Copy,
                    )

            # ---- h_aug = x @ wta (bf16); JF = [F1|J]; hplus strided copy ----
            jf_tiles = {}

            def emit_haug(nt):
                psh = ps_tile([P, WC], f"psh_{nt}")
                for kt in range(KT):
                    pe(nc.tensor.matmul(
                        psh[:],
                        xt_sb[:, kt * N + nt * P : kt * N + (nt + 1) * P],
                        wta_mov(kt),
                        start=(kt == 0),
                        stop=(kt == KT - 1),
                    ))
                if nt not in jf_tiles:
                    jf = jfp.tile([P, 2 * H], F32, tag="jf", name=f"jf_{nt}")
                    nc.scalar.activation(jf[:], psh[:, D : D + 2 * H], AF.Exp)
                    jf_tiles[nt] = jf
                nc.scalar.activation(
                    hp4[:, nt, :, 0:HD],
                    psh[:, 0:D].rearrange("p (h c) -> p h c", h=H),
                    AF.Copy,
                )

            def emit_jf_early(nt):
                # jf only needs the 2H score columns of psh -- a ~190ns tiny
                # matmul per kt instead of waiting on the full h_aug
                psj = ps_tile([P, 2 * H], f"psj_{nt}")
                for kt in range(KT):
                    pe(nc.tensor.matmul(
                        psj[:],
                        xt_sb[:, kt * N + nt * P : kt * N + (nt + 1) * P],
                        wta_all[:, kt * WC + D : kt * WC + D + 2 * H],
                        start=(kt == 0),
                        stop=(kt == KT - 1),
                    ))
                jf = jfp.tile([P, 2 * H], F32, tag="jf", name=f"jf_{nt}")
                nc.scalar.activation(jf[:], psj[:], AF.Exp)
                jf_tiles[nt] = jf

            # critical order for the first TS/TT of pair 0 (heads 0/1, jt 0/1):
            # pss -> e2rep h0/h1 (copies on ACT, keeping the DVE queue clear
            # for the first TS); jf0/jf1 via tiny early matmuls; the full
            # h_aug (for hplus) follows
            emit_e2rep(0)
            emit_jf_early(0)
            emit_jf_early(1)
            emit_e2rep(1)
            # pre-produce jf for nt 2..7 too: the tiny matmuls are ~2x100ns
            # on a warm PE, and pulling the exps ahead of the bulky hp/e2rep
            # ACT ops keeps the DVE from stalling on jf supply early in the
            # main loop (measured ~4us of stretched iterations there)
            for nt in range(2, 8):
                emit_jf_early(nt)
            emit_haug(0)
            emit_haug(1)
            emit_haug(2)
            emit_haug(3)
            emit_e2rep(2)
            emit_e2rep(3)
            for nt in range(4, NT):
                emit_haug(nt)

            # ---- main: P^T construction (DVE) + aggregation + epilogue ----
            ost = bigp.tile([P, ISUB * D], F32, tag="ost")
            ost8 = ost[:].rearrange("p (s c) -> p s c", s=ISUB)

            # epilogue for one (h, half) combo (pair-0 / mid-kernel style:
            # ACT-heavy, stays off the DVE critical path)
            def emit_epilogue(pair, h01, half, psoT):
                h = 2 * pair + h01
                soT = sotp.tile([HP1, 512], F32, tag="soT", name=f"soT_{h}_{half}")
                nc.scalar.activation(soT[:], psoT[:], AF.Copy)
                ps2 = psall.tile(
                    [P, H * HP1], F32, tag=f"bank{h01 * 2 + half}",
                    name=f"ps2_{h}_{half}",
                )
                for q in range(4):
                    pe(nc.tensor.transpose(
                        ps2[:, q * HP1 : (q + 1) * HP1],
                        soT[:, q * P : (q + 1) * P],
                        ident[0:HP1, 0:HP1],
                    ))
                ps2q = ps2[:].rearrange("p (q c) -> p q c", q=4)
                rec4 = smallp.tile([P, 4], F32, tag="rec", name=f"rec_{h}_{half}")
                nc.vector.reciprocal(rec4[:], ps2q[:, :, HD])
                for q in range(4):
                    nc.scalar.activation(
                        ost8[:, half * 4 + q, h * HD : (h + 1) * HD],
                        ps2q[:, q, 0:HD],
                        AF.Copy,
                        scale=rec4[:, q : q + 1],
                    )

            pending_epi = []
            for pair in range(2):
                # pair0 accumulates on banks 0-3, pair1 on banks 4-7 (the
                # h_aug/pss transients are done by then): decoupling the
                # banks lets pair0's epilogues (ps2 on banks 0-3) spread
                # across pair1's first iterations instead of wedging between
                # the pairs, where the DVE reciprocals stalled the stream.
                while pending_epi and pair == 0:
                    emit_epilogue(*pending_epi.pop(0))
                psoTs = {}
                for h01 in range(2):
                    for half in range(2):
                        psoTs[(h01, half)] = psall.tile(
                            [HP1, 512], F32,
                            tag=f"bank{4 * pair + h01 * 2 + half}",
                            name=f"psoT_{2 * pair + h01}_{half}",
                        )
                for jt in range(JT):
                    if pending_epi and jt in (2, 4, 6, 8):
                        emit_epilogue(*pending_epi.pop(0))
                    adjv = adjts[jt][:]
                    # v = max(e2rep*J, F1) per head; pt = v*adj (merged 2-head)
                    v2 = vp.tile([P, 2 * NI], BF16, tag="v", name=f"v_{pair}_{jt}")
                    for h01 in range(2):
                        h = 2 * pair + h01
                        nc.vector.tensor_scalar(
                            v2[:, h01 * NI : (h01 + 1) * NI],
                            e2rep[:, h * NI : (h + 1) * NI],
                            jf_tiles[jt][:, H + h : H + h + 1],
                            jf_tiles[jt][:, h : h + 1],
                            ALU.mult,
                            ALU.max,
                        )
                    pt2 = ptp.tile([P, 2 * NI], BF16, tag="pt", name=f"pt_{pair}_{jt}")
                    nc.vector.tensor_tensor(
                        pt2[:].rearrange("p (g c) -> p g c", g=2),
                        v2[:].rearrange("p (g c) -> p g c", g=2),
                        adjv.unsqueeze(1).broadcast_to([P, 2, NI]),
                        ALU.mult,
                    )
                    # jt-major aggregation: 4 interleaved accumulation groups
                    # (verified on HW: per-cell has_written bits make
                    # interleaved groups on different banks safe)
                    for h01 in range(2):
                        h = 2 * pair + h01
                        for half in range(2):
                            pe(nc.tensor.matmul(
                                psoTs[(h01, half)][:],
                                hplus[:, jt * HPW + h * HP1 : jt * HPW + (h + 1) * HP1],
                                pt2[:, h01 * NI + half * 512 : h01 * NI + (half + 1) * 512],
                                start=(jt == 0),
                                stop=(jt == JT - 1),
                                skip_group_check=True,
                            ))
                for half in range(2):
                    for h01 in range(2):
                        pending_epi.append((pair, h01, half, psoTs[(h01, half)]))

            # ---- tail: pair-1 epilogues, restructured for minimum latency:
            # all soT copies first (split ACT/DVE so they run in parallel),
            # then the 16 transposes stream back-to-back on PE, then DVE
            # reciprocal + DVE normalize (DVE is idle in the tail), with the
            # output DMAs interleaved per half.
            tail = [pending_epi.pop(0) for _ in range(4)]
            soTs = {}
            for k, (pair, h01, half, psoT) in enumerate(tail):
                h = 2 * pair + h01
                soT = sotp.tile(
                    [HP1, 512], F32, tag="soT", name=f"soTt_{h}_{half}"
                )
                if k % 2 == 0:
                    nc.scalar.activation(soT[:], psoT[:], AF.Copy)
                else:
                    nc.vector.tensor_copy(soT[:], psoT[:])
                soTs[(h01, half)] = soT
            ps2s = {}
            for pair, h01, half, psoT in tail:
                h = 2 * pair + h01
                ps2 = psall.tile(
                    [P, H * HP1], F32, tag=f"bank{h01 * 2 + half}",
                    name=f"ps2t_{h}_{half}",
                )
                for q in range(4):
                    pe(nc.tensor.transpose(
                        ps2[:, q * HP1 : (q + 1) * HP1],
                        soTs[(h01, half)][:, q * P : (q + 1) * P],
                        ident[0:HP1, 0:HP1],
                    ))
                ps2s[(h01, half)] = ps2[:].rearrange("p (q c) -> p q c", q=4)
            done_half = set()
            for pair, h01, half, psoT in tail:
                h = 2 * pair + h01
                ps2q = ps2s[(h01, half)]
                rec4 = smallp.tile([P, 4], F32, tag="rec", name=f"rect_{h}_{half}")
                nc.vector.reciprocal(rec4[:], ps2q[:, :, HD])
                nc.vector.tensor_tensor(
                    ost8[:, half * 4 : half * 4 + 4, h * HD : (h + 1) * HD],
                    ps2q[:, :, 0:HD],
                    rec4[:].unsqueeze(2).broadcast_to([P, 4, HD]),
                    ALU.mult,
                )
                key = (h01, half)
                done_half.add(key)
                # once both heads of a half are normalized, its 4 i-subtiles
                # are complete -> one merged output DMA per half (a dma_start
                # issue costs ~700ns of queue time; 2 beats 8, and splitting
                # transfers across queues measured slightly worse)
                if (1 - h01, half) in done_half:
                    eng = nc.sync if half == 0 else nc.gpsimd
                    eng.dma_start(
                        outs[half * 512 : (half + 1) * 512, :].rearrange(
                            "(s p) c -> p s c", s=4
                        ),
                        ost[:, half * 4 * D : (half + 1) * 4 * D].rearrange(
                            "p (s c) -> p s c", s=4
                        ),
                    )

    _split_waits(nc)
    nc.finalize()
    return nc


_NC_CACHE = None


def _get_nc():
    global _NC_CACHE
    if _NC_CACHE is None:
        _NC_CACHE = build_nc()
    return _NC_CACHE


def make_in_maps(x, adj, W, a_src, a_dst):
    x = np.ascontiguousarray(x, dtype=np.float32)
    W = np.ascontiguousarray(W, dtype=np.float32)
    a_src = np.ascontiguousarray(a_src, dtype=np.float32)
    a_dst = np.ascontiguousarray(a_dst, dtype=np.float32)

    A_src = np.zeros((D, H), np.float32)
    A_dst = np.zeros((D, H), np.float32)
    for h in range(H):
        A_src[h * HD : (h + 1) * HD, h] = a_src[h]
        A_dst[h * HD : (h + 1) * HD, h] = a_dst[h]
    Wt = W.T.astype(np.float32)
    wd = Wt @ A_dst
    wta = np.concatenate(
        [Wt, wd, NEG_SLOPE * wd, Wt @ A_src], axis=1
    ).astype(ml_dtypes.bfloat16)

    selm = np.zeros((H, H * P), ml_dtypes.bfloat16)
    for h in range(H):
        selm[h, h * P : (h + 1) * P] = 1.0
    in_maps = []
    adjT_cache = {}
    for c in range(NCORES):
        b, ihalf = c // 2, c % 2
        ilo = ihalf * NI
        if b not in adjT_cache:
            adjT_cache[b] = adj[b].astype(ml_dtypes.bfloat16).T
        # rotate x columns (and adj rows to match) so this core's i-half is
        # always xbt cols 0:NI -- the first xt chunk then feeds pss, jf0-7
        # and h_aug at once, and the duplicate 0.5MB xit load disappears
        xbt_c = np.roll(x[b].T.astype(ml_dtypes.bfloat16), -ilo, axis=1)
        adjt_c = np.roll(adjT_cache[b][:, ilo : ilo + NI], -ilo, axis=0)
        in_maps.append(
            {
                "xbt": np.ascontiguousarray(xbt_c),
                "wta": np.ascontiguousarray(wta),
                "adjtb": np.ascontiguousarray(adjt_c),
                "selm": selm,
            }
        )
    return in_maps


def kernel(x, adj, W, a_src, a_dst):
    in_maps = make_in_maps(x, adj, W, a_src, a_dst)
    nc = _get_nc()
    res = run_bass_kernel_spmd(nc, in_maps, list(range(NCORES)))

    out = np.empty((B, N, D), np.float32)
    for c in range(NCORES):
        b, ihalf = c // 2, c % 2
        ilo = ihalf * NI
        out[b, ilo : ilo + NI, :] = res.results[c]["outs"]
    return out
